# revision 36
# baseline (speedup 1.0000x reference)
"""Multi-head attention (B=2, S=2048, D=1024, H=16, causal) on 8 TRN2 NeuronCores.

Sharding: core c -> (batch b = c//4, head-group g = c%4, heads 4g..4g+3).
Each core computes Q/K/V projections for its 4 heads, causal flash-style
attention, and a partial output projection (its 256 d_model columns of the
ctx @ wo.T contraction).  Host sums the 4 partials per batch and adds bo.

Device layout: "transposed" activations (feature dim on SBUF partitions,
sequence on the free axis) so every matmul contraction runs along
partitions; host passes x.T and pre-transposed weight slices so all DMAs
are contiguous.  Softmax is unnormalized with a fused ones-column in the v
projection (zero weights + bias 1) so the ctx matmul also emits the
denominator.

v2 changes vs the 187-194us baseline (trace-driven):
- Input DMA restructured: per-tensor mega-DMAs (3D access patterns) into
  persistent SBUF tiles, issued in consumption order (v0, v1, q, k) so
  the DMA data queue never reorders ahead of the consumer and the sync
  queue issues ~10 DMA instructions instead of ~60 (DMA_DIRECT2D issue
  cost was ~0.6us each on the in-order sync queue).
- Weight/bias loads are one DMA each via "(d p) f -> p d f" views.
- Triangular-mask muls: one 3D-AP multiply per diagonal k-tile (covers
  both heads) on GpSimd, which is otherwise idle -- the DVE queue no
  longer sits between exp (ACT) and the ctx matmuls (PE).
- Denominator: reciprocal_approx_fast reads the PSUM row directly
  (drops a [1,1024] DVE copy per chunk).
- k-projection is split by feature half (t) with attention chunks
  hp0-j0/j1 emitted between the halves: their exps (ACT) overlap the
  t=1 projection matmuls (PE) and the old 4.8us transition gap is gone.
- Output projection sp=0 drips (a,b)-paired units into hp1-j2/j3 PE
  slack every other k-tile; the sp=1 half runs sh-split (all 512-col
  a-units, then the normalize flush, then all b-units) so the last
  chunk's normalize chain is covered by ~3.4us of independent matmuls.

Measured on 8xTRN2 (axon): baseline 187-194us, rel err 3.6e-3.
"""

import sys

for _p in ("/opt/trn_rl_repo",):
    if _p not in sys.path:
        sys.path.insert(0, _p)

import ml_dtypes
import numpy as np

import concourse.bass as bass
import concourse.mybir as mybir
import concourse.tile as tile
from concourse import bacc, bass_utils

F32 = mybir.dt.float32
F32R = mybir.dt.float32r
BF16 = mybir.dt.bfloat16
AF = mybir.ActivationFunctionType
ALU = mybir.AluOpType

N_CORES = 8
B, S, D, H = 2, 2048, 1024, 16
HG = 4              # heads per core
DK = 64             # head dim
F = HG * DK         # 256 features per core
FA = HG * (DK + 1)  # 260: v features + per-head denominator ones-column
DT = D // 128       # 8 d-tiles
FT = F // 128       # 2 f-tiles
ST = S // 128       # 16 s-tiles / k-tiles


def _build():
    nc = bacc.Bacc("TRN2", target_bir_lowering=False, debug=False,
                   num_devices=N_CORES)

    def din(name, shape, dt=F32):
        return nc.dram_tensor(name, shape, dt, kind="ExternalInput").ap()

    xqT = din("xqT", (D, S), BF16)
    xkT = din("xkT", (D, S), BF16)
    xvT = din("xvT", (D, S), BF16)
    wqT = din("wqT", (D, F), BF16)
    wkT = din("wkT", (D, F), BF16)
    wvT = din("wvT", (D, FA), BF16)   # interleaved, zero ones-columns
    woT = din("woT", (F, D), BF16)
    bq8 = din("bq8", (F, 1))
    bk = din("bk", (F, 1))
    bv260 = din("bv260", (128, FA))  # bv broadcast, 1.0 at ones-columns
    outT = nc.dram_tensor("outT", (D, S), BF16, kind="ExternalOutput").ap()

    with tile.TileContext(nc) as tc:
        with (
            tc.tile_pool(name="const", bufs=1) as cp,
            tc.tile_pool(name="data", bufs=1) as dp,
            tc.tile_pool(name="io", bufs=1) as iop,
            tc.tile_pool(name="dr", bufs=1, space="DRAM") as drp,
            tc.tile_pool(name="pp", bufs=2, space="PSUM") as pp,
        ):
            def psum_sc():
                # scores / general tag: 3 bufs x 2 banks = 6 banks.
                # Depth 3 matters: at depth 2 the ACT exp's retire latency
                # enters the scores critical loop (+3us/long chunk).
                return pp.tile([128, 1024], F32, name="sc", bufs=3)

            def psum_pc():
                # ctx-accumulator tag: 1 buf x 2 banks
                return pp.tile([128, 1024], F32, name="pc", bufs=1)

            # ---- PE warmup: dep-free matmuls so HAM un-throttles the PE
            # clock (K=4/8 -> 8/8) before the first real matmul ----
            wmup = cp.tile([128, 512], BF16, name="wmup")
            nc.gpsimd.memset(wmup[:], 0.0)

            pw = psum_sc()
            for i in range(22):
                nc.tensor.matmul(pw[:, 0:512], wmup[:, 0:128], wmup[:],
                                 start=(i == 0), stop=True,
                                 skip_group_check=True)

            # ---- DMAs: issued in consumption order (xv0+wv first, then
            # xv1, xq, xk, with each phase's weights right after the x it
            # follows in the data queue).  Data-queue order == issue order
            # == consumption order, so the PE trails the stream closely.
            bv_t = cp.tile([128, FA], F32, name="bv")
            nc.sync.dma_start(bv_t[:], bv260[:])
            wv_all = cp.tile([128, DT * FA], BF16, name="wv_all")
            nc.sync.dma_start(
                wv_all[:].rearrange("p (d f) -> p d f", f=FA),
                wvT.rearrange("(d p) f -> p d f", p=128))
            xv0 = dp.tile([128, DT * 1024], BF16, name="xv0")
            for h4 in range(2):   # split: v0's d0 matmuls start ~5us sooner
                nc.sync.dma_start(
                    xv0[:, h4 * 4 * 1024:(h4 + 1) * 4 * 1024].rearrange(
                        "p (d s) -> p d s", s=1024),
                    xvT[h4 * 512:(h4 + 1) * 512, 0:1024].rearrange(
                        "(d p) s -> p d s", p=128))
            xv1 = dp.tile([128, DT * 1024], BF16, name="xv1")
            for h4 in range(2):
                nc.sync.dma_start(
                    xv1[:, h4 * 4 * 1024:(h4 + 1) * 4 * 1024].rearrange(
                        "p (d s) -> p d s", s=1024),
                    xvT[h4 * 512:(h4 + 1) * 512, 1024:2048].rearrange(
                        "(d p) s -> p d s", p=128))
            xq_all = dp.tile([128, DT * S], BF16, name="xq_all")
            for h4 in range(2):   # split for landing granularity
                nc.sync.dma_start(
                    xq_all[:, h4 * 4 * S:(h4 + 1) * 4 * S].rearrange(
                        "p (d s) -> p d s", s=S),
                    xqT[h4 * 512:(h4 + 1) * 512, :].rearrange(
                        "(d p) s -> p d s", p=128))
            wq_all = cp.tile([128, DT * F], BF16, name="wq_all")
            nc.sync.dma_start(
                wq_all[:].rearrange("p (d f) -> p d f", f=F),
                wqT.rearrange("(d p) f -> p d f", p=128))
            bq2 = cp.tile([128, FT], F32, name="bq2")
            nc.sync.dma_start(
                bq2[:].rearrange("p (t o) -> p t o", o=1),
                bq8.rearrange("(t p) o -> p t o", p=128))
            xk_all = dp.tile([128, DT * S], BF16, name="xk_all")
            for h4 in range(2):
                nc.sync.dma_start(
                    xk_all[:, h4 * 4 * S:(h4 + 1) * 4 * S].rearrange(
                        "p (d s) -> p d s", s=S),
                    xkT[h4 * 512:(h4 + 1) * 512, :].rearrange(
                        "(d p) s -> p d s", p=128))
            wk_all = cp.tile([128, DT * F], BF16, name="wk_all")
            nc.sync.dma_start(
                wk_all[:].rearrange("p (d f) -> p d f", f=F),
                wkT.rearrange("(d p) f -> p d f", p=128))
            bk2 = cp.tile([128, FT], F32, name="bk2")
            nc.sync.dma_start(
                bk2[:].rearrange("p (t o) -> p t o", o=1),
                bk.rearrange("(t p) o -> p t o", p=128))
            wo_all = cp.tile([128, FT * D], BF16, name="wo_all")
            nc.sync.dma_start(
                wo_all[:].rearrange("p (t f) -> p t f", f=D),
                woT.rearrange("(t p) f -> p t f", p=128))

            def wv_t(d):
                return wv_all[:, d * FA:(d + 1) * FA]

            def wq_t(d):
                return wq_all[:, d * F:(d + 1) * F]

            def wk_t(d):
                return wk_all[:, d * F:(d + 1) * F]

            def wo_t(t):
                return wo_all[:, t * D:(t + 1) * D]

            # ones row for the last chunk's K=1 partition-broadcast matmul
            ones = cp.tile([1, 128], BF16, name="ones")
            nc.gpsimd.memset(ones[:], 1.0)

            # tri2[p, (h y)] = 1 if y >= p else 0, duplicated for 2 heads:
            # one 3D-AP multiply masks both heads' diagonal blocks
            tri2 = cp.tile([128, 256], BF16, name="tri2")
            nc.gpsimd.memset(tri2[:], 1.0)
            for hh in range(2):
                nc.gpsimd.affine_select(
                    out=tri2[:, hh * 128:(hh + 1) * 128],
                    in_=tri2[:, hh * 128:(hh + 1) * 128],
                    compare_op=ALU.is_ge,
                    fill=0.0, base=0, pattern=[[1, 128]],
                    channel_multiplier=-1)

            # ---- persistent per-core tensors -----------------------------
            qpT = [dp.tile([128, S], BF16, name=f"qpT{t}") for t in range(FT)]
            kpT = [dp.tile([128, S], BF16, name=f"kpT{t}") for t in range(FT)]
            vp = [dp.tile([128, FA], BF16, name=f"vp{st}") for st in range(ST)]
            ctxn = [dp.tile([128, S], BF16, name=f"ctxn{t}") for t in range(FT)]

            # ---- phase A: v projection FIRST  [s, f] natural + ones col --
            def v_half(half):
                xsrc = xv0 if half == 0 else xv1
                pv4 = [psum_sc(), psum_sc(), psum_sc(), psum_pc()]
                pv = {s8: pv4[s8 // 2][:, (s8 % 2) * 512:(s8 % 2) * 512 + FA]
                      for s8 in range(8)}
                for d in range(DT):
                    xd = xsrc[:, d * 1024:(d + 1) * 1024]
                    for s8 in range(8):
                        nc.tensor.matmul(
                            pv[s8],
                            xd[:, s8 * 128:(s8 + 1) * 128],
                            wv_t(d),
                            start=(d == 0), stop=(d == DT - 1))
                        if d == DT - 1:
                            # bias-add right behind this stream's stop so
                            # the adds overlap the remaining streams' mms
                            st = half * 8 + s8
                            nc.vector.tensor_add(vp[st][:], pv[s8], bv_t[:])

            v_half(0)
            v_half(1)

            # ---- phase A: q projection  [f, s] = wT.T @ xT ---------------
            # (1,1) bias first: it owns the pc-tagged psum tile the next
            # phase's pc allocation waits on.  Adds alternate DVE/ACT.
            q4 = [psum_sc(), psum_sc(), psum_sc(), psum_pc()]
            qstreams = {(0, 0): q4[0], (0, 1): q4[1],
                        (1, 0): q4[2], (1, 1): q4[3]}

            def q_add(t, sp, i):
                dst = qpT[t][:, sp * 1024:(sp + 1) * 1024]
                pslice = qstreams[(t, sp)][:]
                bias = bq2[:, t:t + 1]
                if i % 2 == 0:
                    nc.vector.tensor_scalar(
                        dst, pslice, 0.125, bias, op0=ALU.mult, op1=ALU.add)
                else:
                    nc.scalar.activation(
                        dst, pslice, AF.Identity, bias=bias, scale=0.125)

            for d in range(DT):
                xd = xq_all[:, d * S:(d + 1) * S]
                if d < DT - 1:
                    for t in range(FT):
                        lhsT = wq_t(d)[:, t * 128:(t + 1) * 128]
                        for sp in range(2):
                            for sh in range(2):
                                s = sp * 2 + sh
                                nc.tensor.matmul(
                                    qstreams[(t, sp)][:, sh * 512:(sh + 1) * 512],
                                    lhsT,
                                    xd[:, s * 512:(s + 1) * 512],
                                    start=(d == 0), stop=False)
                else:
                    for i, (t, sp) in enumerate(
                            ((1, 1), (0, 0), (0, 1), (1, 0))):
                        lhsT = wq_t(d)[:, t * 128:(t + 1) * 128]
                        for sh in range(2):
                            s = sp * 2 + sh
                            nc.tensor.matmul(
                                qstreams[(t, sp)][:, sh * 512:(sh + 1) * 512],
                                lhsT,
                                xd[:, s * 512:(s + 1) * 512],
                                start=False, stop=True)
                        q_add(t, sp, i)

            # ---- phase A': k projection, split by feature half t, with
            # attention chunks interleaved between the halves -------------
            def k_stream(t):
                kp2 = [psum_sc(), psum_sc()]   # sp = 0, 1
                for d in range(DT):
                    xd = xk_all[:, d * S:(d + 1) * S]
                    lhsT = wk_t(d)[:, t * 128:(t + 1) * 128]
                    last = d == DT - 1
                    for sp in range(2):
                        for sh in range(2):
                            s = sp * 2 + sh
                            nc.tensor.matmul(
                                kp2[sp][:, sh * 512:(sh + 1) * 512],
                                lhsT,
                                xd[:, s * 512:(s + 1) * 512],
                                start=(d == 0), stop=last)
                        if last:
                            dst = kpT[t][:, sp * 1024:(sp + 1) * 1024]
                            bias = bk2[:, t:t + 1]
                            if sp == 0:
                                # halves: the first attention scores only
                                # need cols 0:512, gate them on less work
                                nc.vector.tensor_scalar_add(
                                    dst[:, 0:512], kp2[sp][:, 0:512], bias)
                                nc.vector.tensor_scalar_add(
                                    dst[:, 512:1024], kp2[sp][:, 512:1024],
                                    bias)
                            else:
                                nc.scalar.add(dst, kp2[sp][:], bias)

            # ---- phase B: attention, 512-wide q chunks -------------------
            # Denominator: reciprocal_approx_fast reads the PSUM row
            # directly, then a DRAM round-trip DMA broadcasts it across
            # partitions.  The two normalize multiplies are deferred to the
            # NEXT chunk.  The final chunk broadcasts with a K=1
            # ones-matmul into PSUM instead (PE is idle by then).
            pending_fin = []

            def flush_fin():
                while pending_fin:
                    pending_fin.pop(0)()

            def attn_j(hp, j, inject=None, pe_fin=False):
                    t = hp
                    h0, h1 = 2 * hp, 2 * hp + 1
                    q0 = j * 512
                    pc = psum_pc()              # bank0: head h0, bank1: h1
                    b_started = [False, False]

                    def ctx_mm(kt, ex, c0, last):
                        w = 512 - c0
                        for half, h in ((0, h0), (1, h1)):
                            nc.tensor.matmul(
                                pc[0:65, half * 512 + c0:half * 512 + 512],
                                vp[kt][:, 65 * h:65 * h + 65],
                                ex[:, half * 512:half * 512 + w],
                                start=not b_started[half], stop=last)
                            b_started[half] = True

                    kts = list(range(4 * j + 4))
                    pending = []
                    for idx, kt in enumerate(kts):
                        c0 = max(0, 128 * kt - q0)
                        w = 512 - c0
                        psc = psum_sc()
                        for half, off in ((0, 0), (1, 64)):
                            nc.tensor.matmul(
                                psc[:, half * 512:half * 512 + w],
                                kpT[t][off:off + 64,
                                       kt * 128:(kt + 1) * 128],
                                qpT[t][off:off + 64, q0 + c0:q0 + 512],
                                start=True, stop=True)
                        if idx == 3:
                            # previous chunk's normalize multiplies (PE
                            # broadcast + 2 DVE muls); this chunk's early
                            # work is already queued ahead
                            flush_fin()
                        if inject is not None and idx >= 4:
                            inject(idx)
                        ex = iop.tile([128, 1024], BF16, name="ex", bufs=8)
                        if w == 512:
                            nc.scalar.activation(ex[:], psc[:], AF.Exp)
                        else:
                            v2 = psc[:].rearrange("p (b c) -> p b c",
                                                  c=512)[:, :, 0:w]
                            e2 = ex[:].rearrange("p (b c) -> p b c",
                                                 c=512)[:, :, 0:w]
                            nc.scalar.activation(e2, v2, AF.Exp)
                        if 128 * kt >= q0:   # diagonal: triangular mask --
                            # one 3D-AP mul covers both heads, on GpSimd
                            # (idle engine; keeps DVE out of the exp->ctx
                            # chain; DVE variant measured worse -- it
                            # queues behind the previous chunk's epilogue)
                            e3 = ex[:].rearrange("p (b c) -> p b c",
                                                 c=512)[:, :, 0:128]
                            t3 = tri2[:].rearrange("p (b c) -> p b c",
                                                   c=128)
                            nc.gpsimd.tensor_mul(e3, e3, t3)
                        pending.append((kt, ex, c0))
                        if len(pending) > 2:
                            ctx_mm(*pending.pop(0), last=False)
                    while pending:
                        ctx_mm(*pending.pop(0), last=(len(pending) == 0))

                    # chunk epilogue, all DVE (ACT must stay exp-pure: any
                    # ACT op here idles its in-order queue ~2us/boundary).
                    # Casts FIRST: they plus the dencopy gate the pc psum
                    # release for the next chunk's ctx.  The den row must
                    # stage through SBUF: reciprocal_approx_fast reading
                    # PSUM directly is silently wrong on HW.
                    nc.vector.tensor_copy(
                        ctxn[t][0:64, q0:q0 + 512], pc[0:64, 0:512])
                    nc.vector.tensor_copy(
                        ctxn[t][64:128, q0:q0 + 512], pc[0:64, 512:1024])
                    dst = iop.tile([1, 1024], F32, name="denst", bufs=2)
                    nc.vector.tensor_copy(dst[:], pc[64:65, :])
                    rec = iop.tile([1, 1024], F32, name="rec", bufs=2)
                    nc.vector.reciprocal_approx_fast(rec[:], dst[:])
                    # every chunk broadcasts 1/den across partitions with a
                    # K=1 ones-matmul into PSUM (~0.4us PE) -- the DMA
                    # round-trip variant races the next chunk's fin flush
                    # and stalls DVE when the broadcast lands late
                    rec_b = iop.tile([1, 1024], BF16, name="recb", bufs=2)
                    nc.vector.tensor_copy(rec_b[:], rec[:])

                    def fin(t=t, q0=q0, rec_b=rec_b):
                        bcp = psum_sc()
                        for bh in range(2):
                            nc.tensor.matmul(
                                bcp[:, bh * 512:(bh + 1) * 512],
                                ones[:],
                                rec_b[:, bh * 512:(bh + 1) * 512],
                                start=True, stop=True)
                        nc.vector.tensor_mul(
                            ctxn[t][0:64, q0:q0 + 512],
                            ctxn[t][0:64, q0:q0 + 512],
                            bcp[0:64, 0:512])
                        nc.vector.tensor_mul(
                            ctxn[t][64:128, q0:q0 + 512],
                            ctxn[t][64:128, q0:q0 + 512],
                            bcp[64:128, 512:1024])

                    pending_fin.append(fin)

            # ---- phase C: output projection ------------------------------
            # 32 independent single-shot units (e, sp, sh): two N=512
            # matmuls into half a psum tile, one copy, one store.  The 16
            # sp=0 units drip into the PE slack of the last two attention
            # chunks; sp=1 runs at the end sh-split so the last chunk's
            # normalize chain is covered by the sh=0 units.
            _c_idx = [0]

            def c_pair(e, sp):
                # one (e, sp) pair: 4 matmuls into one psum tile, two
                # copies, ONE [128,1024] store (a DMA instr costs ~0.6us
                # of sync-queue time regardless of size -- 32 small stores
                # were sync-bound).  The phase is copy-bound: DVE also
                # carries the last chunk's epilogue + fins early on, so
                # the first pairs put both copies on ACT.
                po = psum_sc()
                # bufs=6: each buf is held until its store's DMA completes
                # (+0.9us sem propagation); 3 bufs capped the pair cadence
                ob2 = iop.tile([128, 1024], BF16, name="ob2", bufs=6)
                for sh in range(2):
                    c0 = sp * 1024 + sh * 512
                    for t in range(FT):
                        nc.tensor.matmul(
                            po[:, sh * 512:sh * 512 + 512],
                            wo_t(t)[:, e * 128:(e + 1) * 128],
                            ctxn[t][:, c0:c0 + 512],
                            start=(t == 0), stop=(t == FT - 1))
                    pi = _c_idx[0]
                    on_act = pi < 4 or (sh + e) % 2 == 0
                    if on_act:
                        nc.scalar.copy(ob2[:, sh * 512:sh * 512 + 512],
                                       po[:, sh * 512:sh * 512 + 512])
                    else:
                        nc.vector.tensor_copy(
                            ob2[:, sh * 512:sh * 512 + 512],
                            po[:, sh * 512:sh * 512 + 512])
                nc.sync.dma_start(
                    outT[e * 128:(e + 1) * 128,
                         sp * 1024:(sp + 1) * 1024], ob2[:])
                _c_idx[0] += 1

            def phase_c():
                # full output projection after attention: PE-bound stream
                # with both copy engines idle by now and the 4MB output
                # DMA overlapped.  No dripping into the ACT-bound
                # attention chunks -- measured net-negative (psum churn +
                # DVE copies inflated j12/j13 by ~8.5us).
                for e in range(DT):
                    c_pair(e, 0)
                    if e == 2:
                        flush_fin()  # last chunk's normalize multiplies
                for e in range(DT):
                    c_pair(e, 1)

            # q-ordered schedule with k interleave: k-t0, attention j0/j1 of
            # hp0 (their ACT exps overlap k-t1's matmuls), k-t1, the rest.
            k_stream(0)
            attn_j(0, 0)
            attn_j(0, 1)
            k_stream(1)
            attn_j(0, 2)
            attn_j(0, 3)
            attn_j(1, 0)
            attn_j(1, 1)
            attn_j(1, 2)
            attn_j(1, 3, pe_fin=True)
            phase_c()

    nc.compile()
    return nc


_NC_CACHE = {}


def _get_nc():
    if "nc" not in _NC_CACHE:
        _NC_CACHE["nc"] = _build()
    return _NC_CACHE["nc"]


def _in_maps(q, k, v, wq, bq, wk, bk, wv, bv, wo):
    maps = []
    xT = {}
    for b in range(B):
        xT[b] = tuple(np.ascontiguousarray(x[b].T).astype(ml_dtypes.bfloat16)
                      for x in (q, k, v))
    per_g = {}
    for g in range(HG):
        sl = slice(g * F, (g + 1) * F)
        # interleave v weights/bias with the denominator ones-column per head
        wv_aug = np.zeros((D, FA), np.float32)
        bv_aug = np.zeros((FA,), np.float32)
        wv_sl = wv[sl, :]
        bv_sl = bv[sl]
        for h in range(HG):
            wv_aug[:, h * 65:h * 65 + 64] = wv_sl[h * 64:(h + 1) * 64, :].T
            bv_aug[h * 65:h * 65 + 64] = bv_sl[h * 64:(h + 1) * 64]
            bv_aug[h * 65 + 64] = 1.0
        per_g[g] = dict(
            wqT=np.ascontiguousarray(wq[sl, :].T).astype(ml_dtypes.bfloat16),
            wkT=np.ascontiguousarray(wk[sl, :].T).astype(ml_dtypes.bfloat16),
            wvT=wv_aug.astype(ml_dtypes.bfloat16),
            woT=np.ascontiguousarray(wo[:, sl].T).astype(ml_dtypes.bfloat16),
            bq8=np.ascontiguousarray((bq[sl] / 8.0).reshape(F, 1)),
            bk=np.ascontiguousarray(bk[sl].reshape(F, 1)),
            bv260=np.ascontiguousarray(np.broadcast_to(bv_aug, (128, FA))),
        )
    for c in range(N_CORES):
        b, g = c // HG, c % HG
        m = dict(xqT=xT[b][0], xkT=xT[b][1], xvT=xT[b][2])
        m.update(per_g[g])
        maps.append(m)
    return maps


def run(inputs, trace=False, tmpdir=None):
    nc = _get_nc()
    q = np.asarray(inputs["q"], np.float32)
    k = np.asarray(inputs["k"], np.float32)
    v = np.asarray(inputs["v"], np.float32)
    maps = _in_maps(q, k, v,
                    np.asarray(inputs["wq"], np.float32),
                    np.asarray(inputs["bq"], np.float32),
                    np.asarray(inputs["wk"], np.float32),
                    np.asarray(inputs["bk"], np.float32),
                    np.asarray(inputs["wv"], np.float32),
                    np.asarray(inputs["bv"], np.float32),
                    np.asarray(inputs["wo"], np.float32))
    kwargs = {}
    if trace:
        kwargs = dict(trace=True, tmpdir=tmpdir)
    res = bass_utils.run_bass_kernel_spmd(
        nc, maps, core_ids=list(range(N_CORES)), **kwargs)
    bo = np.asarray(inputs["bo"], np.float32)
    out = np.empty((B, S, D), np.float32)
    for b in range(B):
        acc = res.results[4 * b]["outT"].astype(np.float32)
        for g in range(1, HG):
            acc += res.results[4 * b + g]["outT"].astype(np.float32)
        out[b] = acc.T + bo
    return out, res


def kernel(**inputs):
    out, _ = run(inputs)
    return out


# revision 40
# speedup vs baseline: 1.1056x; 1.1056x over previous
"""Multi-head attention (B=2, S=2048, D=1024, H=16, causal) on 8 TRN2 NeuronCores.

Sharding: core c -> (batch b = c//4, head-group g = c%4, heads 4g..4g+3).
Each core computes Q/K/V projections for its 4 heads, causal flash-style
attention, and a partial output projection (its 256 d_model columns of the
ctx @ wo.T contraction).  Host sums the 4 partials per batch and adds bo.

Device layout: "transposed" activations (feature dim on SBUF partitions,
sequence on the free axis) so every matmul contraction runs along
partitions; host passes x.T and pre-transposed weight slices so all DMAs
are contiguous.  Softmax is unnormalized with a fused ones-column in the v
projection (zero weights + bias 1) so the ctx matmul also emits the
denominator.

Trace-driven changes vs the 187-194us baseline:
- Input DMA restructured: per-tensor mega-DMAs (3D access patterns) into
  persistent SBUF tiles, issued in consumption order (wv, xv0, xv1, xq,
  xk) so the DMA data queue never reorders ahead of the consumer and the
  sync queue issues ~15 DMA instructions instead of ~60 (DMA_DIRECT2D
  issue costs ~0.6us each on the in-order sync queue).  This removes the
  PE starvation the per-tile loads caused in the q/k phases.
- Weight/bias loads are one DMA each via "(d p) f -> p d f" views.
- Triangular-mask muls: one 3D-AP multiply per diagonal k-tile (covers
  both heads) on GpSimd, which is otherwise idle -- the DVE queue no
  longer sits between exp (ACT) and the ctx matmuls (PE).
- Denominator: still staged PSUM row -> SBUF before the reciprocal.
  (reciprocal_approx_fast reading PSUM directly is silently WRONG on
  hardware while CoreSim computes it fine -- probe-verified.)
- Chunk epilogue is all-DVE: any ACT op there idles ACT's in-order queue
  ~2us per chunk boundary behind the ctx drain (measured).
- k-projection is split by feature half (t) with attention chunks
  hp0-j0/j1 emitted between the halves: their exps (ACT) overlap the
  t=1 projection matmuls (PE) and the old 4.8us transition gap is gone.
- Output projection runs as one post-attention phase of 16 (e, sp)
  pairs: 4 matmuls -> 2 copies (ACT-heavy early while DVE drains the
  last epilogue) -> ONE [128,1024] store each.  Dripping units into the
  ACT-bound attention chunks measured net-negative (psum churn + DVE
  copies, ~+8.5us); 32 small stores were sync-queue-bound.
- PSUM stays sc=3/pc=1: at scores depth 2 the ACT exp retire latency
  enters the critical loop (+3us per long chunk, measured).

Measured on 8xTRN2 (axon): 183-189us (slowest core 189.1us), rel err
3.6e-3, vs baseline 187.7-193.9us on the same harness.
"""

import sys

for _p in ("/opt/trn_rl_repo",):
    if _p not in sys.path:
        sys.path.insert(0, _p)

import ml_dtypes
import numpy as np

import concourse.bass as bass
import concourse.mybir as mybir
import concourse.tile as tile
from concourse import bacc, bass_utils

F32 = mybir.dt.float32
F32R = mybir.dt.float32r
BF16 = mybir.dt.bfloat16
AF = mybir.ActivationFunctionType
ALU = mybir.AluOpType

N_CORES = 8
B, S, D, H = 2, 2048, 1024, 16
HG = 4              # heads per core
DK = 64             # head dim
F = HG * DK         # 256 features per core
FA = HG * (DK + 1)  # 260: v features + per-head denominator ones-column
DT = D // 128       # 8 d-tiles
FT = F // 128       # 2 f-tiles
ST = S // 128       # 16 s-tiles / k-tiles


def _build():
    nc = bacc.Bacc("TRN2", target_bir_lowering=False, debug=False,
                   num_devices=N_CORES)

    def din(name, shape, dt=F32):
        return nc.dram_tensor(name, shape, dt, kind="ExternalInput").ap()

    xqT = din("xqT", (D, S), BF16)
    xkT = din("xkT", (D, S), BF16)
    xvT = din("xvT", (D, S), BF16)
    wqT = din("wqT", (D, F), BF16)
    wkT = din("wkT", (D, F), BF16)
    wvT = din("wvT", (D, FA), BF16)   # interleaved, zero ones-columns
    woT = din("woT", (F, D), BF16)
    bq8 = din("bq8", (F, 1))
    bk = din("bk", (F, 1))
    bv260 = din("bv260", (128, FA))  # bv broadcast, 1.0 at ones-columns
    outT = nc.dram_tensor("outT", (D, S), BF16, kind="ExternalOutput").ap()

    with tile.TileContext(nc) as tc:
        with (
            tc.tile_pool(name="const", bufs=1) as cp,
            tc.tile_pool(name="data", bufs=1) as dp,
            tc.tile_pool(name="io", bufs=1) as iop,
            tc.tile_pool(name="dr", bufs=1, space="DRAM") as drp,
            tc.tile_pool(name="pp", bufs=2, space="PSUM") as pp,
        ):
            def psum_sc():
                # scores / general tag: 3 bufs x 2 banks = 6 banks.
                # Depth 3 matters: at depth 2 the ACT exp's retire latency
                # enters the scores critical loop (+3us/long chunk).
                return pp.tile([128, 1024], F32, name="sc", bufs=3)

            def psum_pc():
                # ctx-accumulator tag: 1 buf x 2 banks
                return pp.tile([128, 1024], F32, name="pc", bufs=1)

            # ---- PE warmup: dep-free matmuls so HAM un-throttles the PE
            # clock (K=4/8 -> 8/8) before the first real matmul ----
            wmup = cp.tile([128, 512], BF16, name="wmup")
            nc.gpsimd.memset(wmup[:], 0.0)

            pw = psum_sc()
            for i in range(22):
                nc.tensor.matmul(pw[:, 0:512], wmup[:, 0:128], wmup[:],
                                 start=(i == 0), stop=True,
                                 skip_group_check=True)

            # ---- DMAs: issued in consumption order (xv0+wv first, then
            # xv1, xq, xk, with each phase's weights right after the x it
            # follows in the data queue).  Data-queue order == issue order
            # == consumption order, so the PE trails the stream closely.
            bv_t = cp.tile([128, FA], F32, name="bv")
            nc.sync.dma_start(bv_t[:], bv260[:])
            wv_all = cp.tile([128, DT * FA], BF16, name="wv_all")
            nc.sync.dma_start(
                wv_all[:].rearrange("p (d f) -> p d f", f=FA),
                wvT.rearrange("(d p) f -> p d f", p=128))
            xv0 = dp.tile([128, DT * 1024], BF16, name="xv0")
            for h4 in range(2):   # split: v0's d0 matmuls start ~5us sooner
                nc.sync.dma_start(
                    xv0[:, h4 * 4 * 1024:(h4 + 1) * 4 * 1024].rearrange(
                        "p (d s) -> p d s", s=1024),
                    xvT[h4 * 512:(h4 + 1) * 512, 0:1024].rearrange(
                        "(d p) s -> p d s", p=128))
            # split: a single all-or-nothing xv1 DMA lands ~24.5us and the
            # v0->v1 wait gap re-triggers the HAM throttle through v1/q
            xv1 = dp.tile([128, DT * 1024], BF16, name="xv1")
            for h4 in range(2):
                nc.sync.dma_start(
                    xv1[:, h4 * 4 * 1024:(h4 + 1) * 4 * 1024].rearrange(
                        "p (d s) -> p d s", s=1024),
                    xvT[h4 * 512:(h4 + 1) * 512, 1024:2048].rearrange(
                        "(d p) s -> p d s", p=128))
            xq_all = dp.tile([128, DT * S], BF16, name="xq_all")
            for h4 in range(2):   # split for landing granularity
                nc.sync.dma_start(
                    xq_all[:, h4 * 4 * S:(h4 + 1) * 4 * S].rearrange(
                        "p (d s) -> p d s", s=S),
                    xqT[h4 * 512:(h4 + 1) * 512, :].rearrange(
                        "(d p) s -> p d s", p=128))
            wq_all = cp.tile([128, DT * F], BF16, name="wq_all")
            nc.sync.dma_start(
                wq_all[:].rearrange("p (d f) -> p d f", f=F),
                wqT.rearrange("(d p) f -> p d f", p=128))
            bq2 = cp.tile([128, FT], F32, name="bq2")
            nc.sync.dma_start(
                bq2[:].rearrange("p (t o) -> p t o", o=1),
                bq8.rearrange("(t p) o -> p t o", p=128))
            xk_all = dp.tile([128, DT * S], BF16, name="xk_all")
            for h4 in range(2):
                nc.sync.dma_start(
                    xk_all[:, h4 * 4 * S:(h4 + 1) * 4 * S].rearrange(
                        "p (d s) -> p d s", s=S),
                    xkT[h4 * 512:(h4 + 1) * 512, :].rearrange(
                        "(d p) s -> p d s", p=128))
            wk_all = cp.tile([128, DT * F], BF16, name="wk_all")
            nc.sync.dma_start(
                wk_all[:].rearrange("p (d f) -> p d f", f=F),
                wkT.rearrange("(d p) f -> p d f", p=128))
            bk2 = cp.tile([128, FT], F32, name="bk2")
            nc.sync.dma_start(
                bk2[:].rearrange("p (t o) -> p t o", o=1),
                bk.rearrange("(t p) o -> p t o", p=128))
            wo_all = cp.tile([128, FT * D], BF16, name="wo_all")
            nc.sync.dma_start(
                wo_all[:].rearrange("p (t f) -> p t f", f=D),
                woT.rearrange("(t p) f -> p t f", p=128))

            def wv_t(d):
                return wv_all[:, d * FA:(d + 1) * FA]

            def wq_t(d):
                return wq_all[:, d * F:(d + 1) * F]

            def wk_t(d):
                return wk_all[:, d * F:(d + 1) * F]

            def wo_t(t):
                return wo_all[:, t * D:(t + 1) * D]

            # ones row for the last chunk's K=1 partition-broadcast matmul
            ones = cp.tile([1, 128], BF16, name="ones")
            nc.gpsimd.memset(ones[:], 1.0)

            # tri2[p, (h y)] = 1 if y >= p else 0, duplicated for 2 heads:
            # one 3D-AP multiply masks both heads' diagonal blocks
            tri2 = cp.tile([128, 256], BF16, name="tri2")
            nc.gpsimd.memset(tri2[:], 1.0)
            for hh in range(2):
                nc.gpsimd.affine_select(
                    out=tri2[:, hh * 128:(hh + 1) * 128],
                    in_=tri2[:, hh * 128:(hh + 1) * 128],
                    compare_op=ALU.is_ge,
                    fill=0.0, base=0, pattern=[[1, 128]],
                    channel_multiplier=-1)

            # ---- persistent per-core tensors -----------------------------
            qpT = [dp.tile([128, S], BF16, name=f"qpT{t}") for t in range(FT)]
            kpT = [dp.tile([128, S], BF16, name=f"kpT{t}") for t in range(FT)]
            vp = [dp.tile([128, FA], BF16, name=f"vp{st}") for st in range(ST)]
            ctxn = [dp.tile([128, S], BF16, name=f"ctxn{t}") for t in range(FT)]

            # ---- phase A: v projection FIRST  [s, f] natural + ones col --
            def v_half(half):
                xsrc = xv0 if half == 0 else xv1
                pv4 = [psum_sc(), psum_sc(), psum_sc(), psum_pc()]
                pv = {s8: pv4[s8 // 2][:, (s8 % 2) * 512:(s8 % 2) * 512 + FA]
                      for s8 in range(8)}
                for d in range(DT):
                    xd = xsrc[:, d * 1024:(d + 1) * 1024]
                    for s8 in range(8):
                        nc.tensor.matmul(
                            pv[s8],
                            xd[:, s8 * 128:(s8 + 1) * 128],
                            wv_t(d),
                            start=(d == 0), stop=(d == DT - 1))
                        if d == DT - 1:
                            # bias-add right behind this stream's stop so
                            # the adds overlap the remaining streams' mms
                            st = half * 8 + s8
                            nc.vector.tensor_add(vp[st][:], pv[s8], bv_t[:])

            v_half(0)
            v_half(1)

            # ---- phase A: q projection  [f, s] = wT.T @ xT ---------------
            # (1,1) bias first: it owns the pc-tagged psum tile the next
            # phase's pc allocation waits on.  Adds alternate DVE/ACT.
            q4 = [psum_sc(), psum_sc(), psum_sc(), psum_pc()]
            qstreams = {(0, 0): q4[0], (0, 1): q4[1],
                        (1, 0): q4[2], (1, 1): q4[3]}

            def q_add(t, sp, i):
                dst = qpT[t][:, sp * 1024:(sp + 1) * 1024]
                pslice = qstreams[(t, sp)][:]
                bias = bq2[:, t:t + 1]
                if i % 2 == 0:
                    nc.vector.tensor_scalar(
                        dst, pslice, 0.125, bias, op0=ALU.mult, op1=ALU.add)
                else:
                    nc.scalar.activation(
                        dst, pslice, AF.Identity, bias=bias, scale=0.125)

            for d in range(DT):
                xd = xq_all[:, d * S:(d + 1) * S]
                if d < DT - 1:
                    for t in range(FT):
                        lhsT = wq_t(d)[:, t * 128:(t + 1) * 128]
                        for sp in range(2):
                            for sh in range(2):
                                s = sp * 2 + sh
                                nc.tensor.matmul(
                                    qstreams[(t, sp)][:, sh * 512:(sh + 1) * 512],
                                    lhsT,
                                    xd[:, s * 512:(s + 1) * 512],
                                    start=(d == 0), stop=False)
                else:
                    for i, (t, sp) in enumerate(
                            ((1, 1), (0, 0), (0, 1), (1, 0))):
                        lhsT = wq_t(d)[:, t * 128:(t + 1) * 128]
                        for sh in range(2):
                            s = sp * 2 + sh
                            nc.tensor.matmul(
                                qstreams[(t, sp)][:, sh * 512:(sh + 1) * 512],
                                lhsT,
                                xd[:, s * 512:(s + 1) * 512],
                                start=False, stop=True)
                        q_add(t, sp, i)

            # ---- phase A': k projection, split by feature half t, with
            # attention chunks interleaved between the halves -------------
            def k_stream(t):
                kp2 = [psum_sc(), psum_sc()]   # sp = 0, 1
                for d in range(DT):
                    xd = xk_all[:, d * S:(d + 1) * S]
                    lhsT = wk_t(d)[:, t * 128:(t + 1) * 128]
                    last = d == DT - 1
                    for sp in range(2):
                        for sh in range(2):
                            s = sp * 2 + sh
                            nc.tensor.matmul(
                                kp2[sp][:, sh * 512:(sh + 1) * 512],
                                lhsT,
                                xd[:, s * 512:(s + 1) * 512],
                                start=(d == 0), stop=last)
                        if last:
                            dst = kpT[t][:, sp * 1024:(sp + 1) * 1024]
                            bias = bk2[:, t:t + 1]
                            if sp == 0:
                                # halves: the first attention scores only
                                # need cols 0:512, gate them on less work
                                nc.vector.tensor_scalar_add(
                                    dst[:, 0:512], kp2[sp][:, 0:512], bias)
                                nc.vector.tensor_scalar_add(
                                    dst[:, 512:1024], kp2[sp][:, 512:1024],
                                    bias)
                            else:
                                nc.scalar.add(dst, kp2[sp][:], bias)

            # ---- phase B: attention, 512-wide q chunks -------------------
            # Denominator: reciprocal_approx_fast reads the PSUM row
            # directly, then a DRAM round-trip DMA broadcasts it across
            # partitions.  The two normalize multiplies are deferred to the
            # NEXT chunk.  The final chunk broadcasts with a K=1
            # ones-matmul into PSUM instead (PE is idle by then).
            pending_fin = []

            def flush_fin():
                while pending_fin:
                    pending_fin.pop(0)()

            def attn_j(hp, j, inject=None, pe_fin=False):
                    t = hp
                    h0, h1 = 2 * hp, 2 * hp + 1
                    q0 = j * 512
                    pc = psum_pc()              # bank0: head h0, bank1: h1
                    b_started = [False, False]

                    def ctx_mm(kt, ex, c0, last):
                        w = 512 - c0
                        for half, h in ((0, h0), (1, h1)):
                            nc.tensor.matmul(
                                pc[0:65, half * 512 + c0:half * 512 + 512],
                                vp[kt][:, 65 * h:65 * h + 65],
                                ex[:, half * 512:half * 512 + w],
                                start=not b_started[half], stop=last)
                            b_started[half] = True

                    kts = list(range(4 * j + 4))
                    pending = []
                    for idx, kt in enumerate(kts):
                        c0 = max(0, 128 * kt - q0)
                        w = 512 - c0
                        psc = psum_sc()
                        for half, off in ((0, 0), (1, 64)):
                            nc.tensor.matmul(
                                psc[:, half * 512:half * 512 + w],
                                kpT[t][off:off + 64,
                                       kt * 128:(kt + 1) * 128],
                                qpT[t][off:off + 64, q0 + c0:q0 + 512],
                                start=True, stop=True)
                        if idx == (3 if len(kts) < 12 else 5):
                            # previous chunk's normalize multiplies: by now
                            # its broadcast DMA has landed AND this chunk's
                            # early work is already queued ahead (idx 5 on
                            # long chunks: more margin for the bc landing)
                            flush_fin()
                        if inject is not None and idx >= 4:
                            inject(idx)
                        ex = iop.tile([128, 1024], BF16, name="ex", bufs=8)
                        if w == 512:
                            nc.scalar.activation(ex[:], psc[:], AF.Exp)
                        else:
                            v2 = psc[:].rearrange("p (b c) -> p b c",
                                                  c=512)[:, :, 0:w]
                            e2 = ex[:].rearrange("p (b c) -> p b c",
                                                 c=512)[:, :, 0:w]
                            nc.scalar.activation(e2, v2, AF.Exp)
                        if 128 * kt >= q0:   # diagonal: triangular mask --
                            # one 3D-AP mul covers both heads, on GpSimd
                            # (idle engine; keeps DVE out of the exp->ctx
                            # chain; DVE variant measured worse -- it
                            # queues behind the previous chunk's epilogue)
                            e3 = ex[:].rearrange("p (b c) -> p b c",
                                                 c=512)[:, :, 0:128]
                            t3 = tri2[:].rearrange("p (b c) -> p b c",
                                                   c=128)
                            nc.gpsimd.tensor_mul(e3, e3, t3)
                        pending.append((kt, ex, c0))
                        if len(pending) > 2:
                            ctx_mm(*pending.pop(0), last=False)
                    while pending:
                        ctx_mm(*pending.pop(0), last=(len(pending) == 0))

                    # chunk epilogue, all DVE (ACT must stay exp-pure: any
                    # ACT op here idles its in-order queue ~2us/boundary).
                    # Casts FIRST: they plus the dencopy gate the pc psum
                    # release for the next chunk's ctx.  The den row must
                    # stage through SBUF: reciprocal_approx_fast reading
                    # PSUM directly is silently wrong on HW.
                    nc.vector.tensor_copy(
                        ctxn[t][0:64, q0:q0 + 512], pc[0:64, 0:512])
                    nc.vector.tensor_copy(
                        ctxn[t][64:128, q0:q0 + 512], pc[0:64, 512:1024])
                    dst = iop.tile([1, 1024], F32, name="denst", bufs=2)
                    nc.vector.tensor_copy(dst[:], pc[64:65, :])
                    rec = iop.tile([1, 1024], F32, name="rec", bufs=2)
                    nc.vector.reciprocal_approx_fast(rec[:], dst[:])
                    if pe_fin:
                        rec_b = iop.tile([1, 1024], BF16, name="recb",
                                         bufs=2)
                        nc.vector.tensor_copy(rec_b[:], rec[:])

                        def fin(t=t, q0=q0, rec_b=rec_b):
                            bcp = psum_sc()
                            for bh in range(2):
                                nc.tensor.matmul(
                                    bcp[:, bh * 512:(bh + 1) * 512],
                                    ones[:],
                                    rec_b[:, bh * 512:(bh + 1) * 512],
                                    start=True, stop=True)
                            nc.vector.tensor_mul(
                                ctxn[t][0:64, q0:q0 + 512],
                                ctxn[t][0:64, q0:q0 + 512],
                                bcp[0:64, 0:512])
                            nc.vector.tensor_mul(
                                ctxn[t][64:128, q0:q0 + 512],
                                ctxn[t][64:128, q0:q0 + 512],
                                bcp[64:128, 512:1024])
                    else:
                        dstage = drp.tile([1, 1024], F32, name="dstage",
                                          bufs=2)
                        nc.sync.dma_start(dstage[:], rec[:])
                        bc = iop.tile([128, 1024], F32, name="bc", bufs=2)
                        nc.sync.dma_start(
                            bc[:], dstage[0:1, :].partition_broadcast(128))

                        def fin(t=t, q0=q0, bc=bc):
                            nc.vector.tensor_mul(
                                ctxn[t][0:64, q0:q0 + 512],
                                ctxn[t][0:64, q0:q0 + 512], bc[0:64, 0:512])
                            nc.vector.tensor_mul(
                                ctxn[t][64:128, q0:q0 + 512],
                                ctxn[t][64:128, q0:q0 + 512],
                                bc[64:128, 512:1024])

                    pending_fin.append(fin)

            # ---- phase C: output projection ------------------------------
            # 32 independent single-shot units (e, sp, sh): two N=512
            # matmuls into half a psum tile, one copy, one store.  The 16
            # sp=0 units drip into the PE slack of the last two attention
            # chunks; sp=1 runs at the end sh-split so the last chunk's
            # normalize chain is covered by the sh=0 units.
            _c_idx = [0]

            def c_pair(e, sp):
                # one (e, sp) pair: 4 matmuls into one psum tile, two
                # copies, ONE [128,1024] store (a DMA instr costs ~0.6us
                # of sync-queue time regardless of size -- 32 small stores
                # were sync-bound).  The phase is copy-bound: DVE also
                # carries the last chunk's epilogue + fins early on, so
                # the first pairs put both copies on ACT.
                po = psum_sc()
                # bufs=6: each buf is held until its store's DMA completes
                # (+0.9us sem propagation); 3 bufs capped the pair cadence
                ob2 = iop.tile([128, 1024], BF16, name="ob2", bufs=6)
                for sh in range(2):
                    c0 = sp * 1024 + sh * 512
                    for t in range(FT):
                        nc.tensor.matmul(
                            po[:, sh * 512:sh * 512 + 512],
                            wo_t(t)[:, e * 128:(e + 1) * 128],
                            ctxn[t][:, c0:c0 + 512],
                            start=(t == 0), stop=(t == FT - 1))
                # ONE [128,1024] copy per pair (halves the ~260ns/copy
                # fixed overhead vs two [128,512] copies), alternating
                # engines per pair; first pairs on ACT while DVE drains
                # the last chunk's epilogue
                pi = _c_idx[0]
                if pi < 4 or pi % 2 == 0:
                    nc.scalar.copy(ob2[:], po[:])
                else:
                    nc.vector.tensor_copy(ob2[:], po[:])
                nc.sync.dma_start(
                    outT[e * 128:(e + 1) * 128,
                         sp * 1024:(sp + 1) * 1024], ob2[:])
                _c_idx[0] += 1

            def phase_c():
                # full output projection after attention: PE-bound stream
                # with both copy engines idle by now and the 4MB output
                # DMA overlapped.  No dripping into the ACT-bound
                # attention chunks -- measured net-negative (psum churn +
                # DVE copies inflated j12/j13 by ~8.5us).
                for e in range(DT):
                    c_pair(e, 0)
                    if e == 2:
                        flush_fin()  # last chunk's normalize multiplies
                for e in range(DT):
                    c_pair(e, 1)

            # q-ordered schedule with k interleave: k-t0, attention j0/j1 of
            # hp0 (their ACT exps overlap k-t1's matmuls), k-t1, the rest.
            k_stream(0)
            attn_j(0, 0)
            attn_j(0, 1)
            k_stream(1)
            attn_j(0, 2)
            attn_j(0, 3)
            attn_j(1, 0)
            attn_j(1, 1)
            attn_j(1, 2)
            attn_j(1, 3, pe_fin=True)
            phase_c()

    nc.compile()
    return nc


_NC_CACHE = {}


def _get_nc():
    if "nc" not in _NC_CACHE:
        _NC_CACHE["nc"] = _build()
    return _NC_CACHE["nc"]


def _in_maps(q, k, v, wq, bq, wk, bk, wv, bv, wo):
    maps = []
    xT = {}
    for b in range(B):
        xT[b] = tuple(np.ascontiguousarray(x[b].T).astype(ml_dtypes.bfloat16)
                      for x in (q, k, v))
    per_g = {}
    for g in range(HG):
        sl = slice(g * F, (g + 1) * F)
        # interleave v weights/bias with the denominator ones-column per head
        wv_aug = np.zeros((D, FA), np.float32)
        bv_aug = np.zeros((FA,), np.float32)
        wv_sl = wv[sl, :]
        bv_sl = bv[sl]
        for h in range(HG):
            wv_aug[:, h * 65:h * 65 + 64] = wv_sl[h * 64:(h + 1) * 64, :].T
            bv_aug[h * 65:h * 65 + 64] = bv_sl[h * 64:(h + 1) * 64]
            bv_aug[h * 65 + 64] = 1.0
        per_g[g] = dict(
            wqT=np.ascontiguousarray(wq[sl, :].T).astype(ml_dtypes.bfloat16),
            wkT=np.ascontiguousarray(wk[sl, :].T).astype(ml_dtypes.bfloat16),
            wvT=wv_aug.astype(ml_dtypes.bfloat16),
            woT=np.ascontiguousarray(wo[:, sl].T).astype(ml_dtypes.bfloat16),
            bq8=np.ascontiguousarray((bq[sl] / 8.0).reshape(F, 1)),
            bk=np.ascontiguousarray(bk[sl].reshape(F, 1)),
            bv260=np.ascontiguousarray(np.broadcast_to(bv_aug, (128, FA))),
        )
    for c in range(N_CORES):
        b, g = c // HG, c % HG
        m = dict(xqT=xT[b][0], xkT=xT[b][1], xvT=xT[b][2])
        m.update(per_g[g])
        maps.append(m)
    return maps


def run(inputs, trace=False, tmpdir=None):
    nc = _get_nc()
    q = np.asarray(inputs["q"], np.float32)
    k = np.asarray(inputs["k"], np.float32)
    v = np.asarray(inputs["v"], np.float32)
    maps = _in_maps(q, k, v,
                    np.asarray(inputs["wq"], np.float32),
                    np.asarray(inputs["bq"], np.float32),
                    np.asarray(inputs["wk"], np.float32),
                    np.asarray(inputs["bk"], np.float32),
                    np.asarray(inputs["wv"], np.float32),
                    np.asarray(inputs["bv"], np.float32),
                    np.asarray(inputs["wo"], np.float32))
    kwargs = {}
    if trace:
        kwargs = dict(trace=True, tmpdir=tmpdir)
    res = bass_utils.run_bass_kernel_spmd(
        nc, maps, core_ids=list(range(N_CORES)), **kwargs)
    bo = np.asarray(inputs["bo"], np.float32)
    out = np.empty((B, S, D), np.float32)
    for b in range(B):
        acc = res.results[4 * b]["outT"].astype(np.float32)
        for g in range(1, HG):
            acc += res.results[4 * b + g]["outT"].astype(np.float32)
        out[b] = acc.T + bo
    return out, res


def kernel(**inputs):
    out, _ = run(inputs)
    return out
